# revision 2
# baseline (speedup 1.0000x reference)
"""Trainium2 Bass kernel for ContextQueryAttention (BiDAF-style), v4.

v3 mixed-precision scheme + off-chip dtype staging:
  - host ships C (f32), Cb=bf16(C), C8=fp8(16C), Qb=bf16(Q), msp, vq
  - CT/QT via XBAR DMA-transpose directly from DRAM bf16 (no PE transposes,
    no PSUM->SBUF copies for them, no on-device converts of C/Q)
  - scores/A in bf16, T/B/c0 in fp8 DoubleRow, P fp8 masked copy for the
    column path; epilogue split across Pool/ACT/DVE.
  - merged DMAs (2-4 m-tiles per transfer) to cut dispatch overhead.
"""
import sys
sys.path.insert(0, "/opt/trn_rl_repo")

import numpy as np
import ml_dtypes
from contextlib import ExitStack

from concourse import bass, bacc, mybir, tile, masks
from concourse.bass_utils import run_bass_kernel_spmd

F32 = mybir.dt.float32
BF16 = mybir.dt.bfloat16
F8 = mybir.dt.float8e4
AF = mybir.ActivationFunctionType
OP = mybir.AluOpType
DR = mybir.MatmulPerfMode.DoubleRow

B, LC, LQ, D = 32, 1024, 256, 512
NCORES = 8
BPC = B // NCORES
MT, JT, KT = LC // 128, LQ // 128, D // 128   # 8, 2, 4
NEGB = -30.0
SP = 0.5        # P fp8 scale
SC8 = 16.0      # C fp8 scale
EPS = 1e-20

_CACHE = {}


def _build():
    nc = bacc.Bacc("TRN2", target_bir_lowering=False, debug=False)
    C_d = nc.dram_tensor("C", [BPC, LC, D], F32, kind="ExternalInput")
    Cb_d = nc.dram_tensor("Cb", [BPC, LC, D], BF16, kind="ExternalInput")
    C8_d = nc.dram_tensor("C8", [BPC, LC, D], F8, kind="ExternalInput")
    Qb_d = nc.dram_tensor("Qb", [BPC, LQ, D], BF16, kind="ExternalInput")
    W_d = nc.dram_tensor("W0", [3 * D], F32, kind="ExternalInput")
    msp_d = nc.dram_tensor("msp", [BPC, LC], F32, kind="ExternalInput")
    vq_d = nc.dram_tensor("vq", [BPC, LQ], F32, kind="ExternalInput")
    out_d = nc.dram_tensor("out", [BPC, LC, 4 * D], F32, kind="ExternalOutput")

    with tile.TileContext(nc) as tc, ExitStack() as ctx:
        const = ctx.enter_context(tc.tile_pool(name="const", bufs=1))
        big = ctx.enter_context(tc.tile_pool(name="big", bufs=2))
        mid = ctx.enter_context(tc.tile_pool(name="mid", bufs=2))
        sm = ctx.enter_context(tc.tile_pool(name="sm", bufs=4))
        pmm = ctx.enter_context(tc.tile_pool(name="pmm", bufs=3, space="PSUM"))
        psc = ctx.enter_context(tc.tile_pool(name="psc", bufs=2, space="PSUM"))
        ptp = ctx.enter_context(tc.tile_pool(name="ptp", bufs=2, space="PSUM"))
        pc0 = ctx.enter_context(tc.tile_pool(name="pc0", bufs=1, space="PSUM"))

        # ---- constants ----
        W_sb = const.tile([128, 12], F32)      # cols 0:4 wc, 4:8 wq, 8:12 wm
        nc.sync.dma_start(W_sb[:], W_d.ap().rearrange("(n p) -> p n", p=128))
        ident_f = const.tile([128, 128], F32)
        masks.make_identity(nc, ident_f[:])
        ident_bf = const.tile([128, 128], BF16)
        nc.vector.tensor_copy(ident_bf[:], ident_f[:])
        ones_f = const.tile([1, 128], F32)
        nc.gpsimd.memset(ones_f[:], 1.0)
        ones_bf = const.tile([1, 128], BF16)
        nc.vector.tensor_copy(ones_bf[:], ones_f[:])
        ones8f = const.tile([128, 32], F32)
        nc.gpsimd.memset(ones8f[:], 1.0)
        ones8 = const.tile([128, 32], F8)
        nc.vector.tensor_copy(ones8[:], ones8f[:])
        ones8_ap = ones8[:, 0:32].rearrange("p (h x) -> p h x", h=2)[:, :, 0:1]

        for b in range(BPC):
            # ---------------- loads (merged) ----------------
            C_sb = big.tile([128, MT * D], F32, tag="C_sb", bufs=2)
            for g in range(2):
                nc.sync.dma_start(
                    C_sb[:, g * 4 * D:(g + 1) * 4 * D]
                    .rearrange("p (m d) -> p m d", m=4),
                    C_d.ap()[b, g * 512:(g + 1) * 512, :]
                    .rearrange("(m p) d -> p m d", p=128))
            C8 = big.tile([128, MT * D], F8, tag="C8", bufs=2)
            for g in range(2):
                nc.sync.dma_start(
                    C8[:, g * 4 * D:(g + 1) * 4 * D]
                    .rearrange("p (m d) -> p m d", m=4),
                    C8_d.ap()[b, g * 512:(g + 1) * 512, :]
                    .rearrange("(m p) d -> p m d", p=128))
            Q_bf = mid.tile([128, JT * D], BF16, tag="Q_bf", bufs=2)
            nc.sync.dma_start(
                Q_bf[:].rearrange("p (j d) -> p j d", j=JT),
                Qb_d.ap()[b].rearrange("(j p) d -> p j d", p=128))
            msp = sm.tile([128, MT], F32, tag="msp")
            nc.gpsimd.dma_start(msp[:], msp_d.ap()[b].rearrange("(m p) -> p m", p=128))
            vqf = sm.tile([1, LQ], F32, tag="vqf")
            nc.gpsimd.dma_start(vqf[:], vq_d.ap()[b].rearrange("(o q) -> o q", o=1))
            vq_bf = sm.tile([1, LQ], BF16, tag="vq_bf")
            nc.vector.tensor_copy(vq_bf[:], vqf[:])

            # ---------------- XBAR transposes from DRAM ----------------
            CT = [big.tile([128, LC], BF16, tag="CT", name=f"CT{_k}", bufs=6)
                  for _k in range(KT)]
            for k in range(KT):
                nc.sync.dma_start(CT[k][:], Cb_d.ap()[b, :, k * 128:(k + 1) * 128],
                                  transpose=True)
            QT = [mid.tile([128, LQ], BF16, tag="QT", name=f"QT{_k}", bufs=6)
                  for _k in range(KT)]
            QW = [mid.tile([128, LQ], BF16, tag="QW", name=f"QW{_k}", bufs=6)
                  for _k in range(KT)]
            for k in range(KT):
                nc.sync.dma_start(QT[k][:], Qb_d.ap()[b, :, k * 128:(k + 1) * 128],
                                  transpose=True)
                nc.vector.tensor_scalar(QW[k][:], QT[k][:],
                                        W_sb[:, 8 + k:9 + k], W_sb[:, k:k + 1],
                                        OP.mult, OP.add)

            # ---------------- scores + exp -> P_bf ----------------
            P_bf = big.tile([128, MT * LQ], BF16, tag="P_bf", bufs=2)
            r_rec = sm.tile([128, MT], F32, tag="r_rec")
            rb_rec = sm.tile([128, MT], F32, tag="rb_rec")
            for m in range(MT):
                ps_S = psc.tile([128, LQ], F32, tag="ps_S")
                for k in range(KT):
                    nc.tensor.matmul(ps_S[:], CT[k][:, m * 128:(m + 1) * 128],
                                     QW[k][:], start=(k == 0), stop=False)
                nc.tensor.matmul(ps_S[:], ones_bf[:], vq_bf[:],
                                 start=False, stop=True, skip_group_check=True)
                r_m = sm.tile([128, 1], F32, tag="r_m", bufs=4)
                nc.scalar.activation(P_bf[:, m * LQ:(m + 1) * LQ], ps_S[:],
                                     AF.Exp, bias=0.0, scale=1.0,
                                     accum_out=r_m[:])
                nc.vector.reciprocal(r_rec[:, m:m + 1], r_m[:])
            nc.vector.tensor_scalar_mul(rb_rec[:], r_rec[:], 1.0 / (SP * 4.0))

            # ---------------- P8m (masked fp8) ----------------
            P8m = big.tile([128, MT * LQ], F8, tag="P8m", bufs=2)
            for m in range(MT):
                nc.vector.tensor_scalar_mul(P8m[:, m * LQ:(m + 1) * LQ],
                                            P_bf[:, m * LQ:(m + 1) * LQ],
                                            msp[:, m:m + 1])

            # ---------------- c0 via DR matvec; T via DR ----------------
            c0q = sm.tile([128, JT], F32, tag="c0q")
            c0r = sm.tile([128, JT], F32, tag="c0r")
            T8 = mid.tile([128, JT * D], F8, tag="T8", bufs=2)
            for jg in range(JT):
                ps_c0 = pc0.tile([128, 1], F32, tag="ps_c0")
                for t in range(MT // 2):
                    lhsT = (P8m[:, t * 512:(t + 1) * 512]
                            .rearrange("p (h j) -> p h j", h=2)
                            [:, :, jg * 128:(jg + 1) * 128])
                    nc.tensor.matmul(ps_c0[:], lhsT, ones8_ap,
                                     start=(t == 0), stop=(t == MT // 2 - 1),
                                     perf_mode=DR)
                # ps_T = SP*SC8*sum(mPC); ps_c0 = SP*sum(mP).
                # T8 = 4*T_true = ps_T * (4/SC8)/ps_c0 -> c0q = 4*ps_c0
                nc.vector.tensor_scalar(c0q[:, jg:jg + 1], ps_c0[:],
                                        4.0, EPS, OP.mult, OP.add)
                nc.vector.reciprocal(c0r[:, jg:jg + 1], c0q[:, jg:jg + 1])
                ps_T = pmm.tile([128, 512], F32, tag="pmm")
                for t in range(MT // 2):
                    lhsT = (P8m[:, t * 512:(t + 1) * 512]
                            .rearrange("p (h j) -> p h j", h=2)
                            [:, :, jg * 128:(jg + 1) * 128])
                    rhs = (C8[:, t * 2 * D:(t + 1) * 2 * D]
                           .rearrange("p (h d) -> p h d", h=2))
                    nc.tensor.matmul(ps_T[:], lhsT, rhs,
                                     start=(t == 0), stop=(t == MT // 2 - 1),
                                     perf_mode=DR)
                nc.vector.tensor_scalar_mul(T8[:, jg * D:(jg + 1) * D],
                                            ps_T[:], c0r[:, jg:jg + 1])

            # ---------------- PT transposes (PE) + PT8 ----------------
            PT = [mid.tile([128, LC], BF16, tag="PT", name=f"PT{_j}", bufs=4)
                  for _j in range(JT)]
            for jg in range(JT):
                for mh in range(2):
                    ps_pt = ptp.tile([128, 512], BF16, tag="ps_tp")
                    for mb in range(4):
                        m = mh * 4 + mb
                        nc.tensor.transpose(
                            ps_pt[:, mb * 128:(mb + 1) * 128],
                            P_bf[:, m * LQ + jg * 128: m * LQ + (jg + 1) * 128],
                            ident_bf[:])
                    nc.vector.tensor_copy(PT[jg][:, mh * 512:(mh + 1) * 512], ps_pt[:])
            PT8 = mid.tile([128, JT * LC], F8, tag="PT8", bufs=2)
            for jg in range(JT):
                nc.vector.tensor_scalar_mul(PT8[:, jg * LC:(jg + 1) * LC],
                                            PT[jg][:], SP)

            # ---------------- A (bf16), Bt (DR), epilogue ----------------
            for t in range(MT // 2):
                o_st = mid.tile([128, 2 * 1536], F32, tag="o_st", bufs=2)
                for g in range(2):
                    m = 2 * t + g
                    o_m = o_st[:, g * 1536:(g + 1) * 1536]
                    ps_A = pmm.tile([128, 512], F32, tag="pmm")
                    for jg in range(JT):
                        nc.tensor.matmul(ps_A[:], PT[jg][:, m * 128:(m + 1) * 128],
                                         Q_bf[:, jg * D:(jg + 1) * D],
                                         start=(jg == 0), stop=(jg == JT - 1))
                    ps_B = pmm.tile([128, 512], F32, tag="pmm")
                    lhsT = (PT8[:].rearrange("p (h i) -> p h i", h=2)
                            [:, :, m * 128:(m + 1) * 128])
                    rhs = T8[:].rearrange("p (h d) -> p h d", h=2)
                    nc.tensor.matmul(ps_B[:], lhsT, rhs, start=True, stop=True,
                                     perf_mode=DR)
                    # A' to o_m[0:512] (DVE/ACT alternating)
                    if m % 2 == 0:
                        nc.vector.tensor_scalar_mul(o_m[:, 0:512], ps_A[:],
                                                    r_rec[:, m:m + 1])
                    else:
                        nc.scalar.activation(o_m[:, 0:512], ps_A[:], AF.Copy,
                                             bias=0.0, scale=r_rec[:, m:m + 1])
                    bt_sb = sm.tile([128, 512], F32, tag="bt_sb", bufs=2)
                    nc.scalar.activation(bt_sb[:], ps_B[:], AF.Copy,
                                         bias=0.0, scale=rb_rec[:, m:m + 1])
                    # epilogue multiplies: C*A' on Pool; C*B' alternates ACT/DVE
                    nc.gpsimd.tensor_tensor(o_m[:, 512:1024],
                                            C_sb[:, m * D:(m + 1) * D],
                                            o_m[:, 0:512], OP.mult)
                    nc.vector.tensor_tensor(o_m[:, 1024:1536],
                                            C_sb[:, m * D:(m + 1) * D],
                                            bt_sb[:], OP.mult)
                # merged stores for the m-pair
                nc.sync.dma_start(
                    out_d.ap()[b, t * 256:(t + 1) * 256, 0:512]
                    .rearrange("(g p) d -> p g d", p=128),
                    C_sb[:, t * 1024:(t + 1) * 1024]
                    .rearrange("p (g d) -> p g d", g=2))
                nc.sync.dma_start(
                    out_d.ap()[b, t * 256:(t + 1) * 256, 512:2048]
                    .rearrange("(g p) c -> p g c", p=128),
                    o_st[:].rearrange("p (g c) -> p g c", g=2))
    nc.compile()
    return nc


def _get_nc():
    if "nc" not in _CACHE:
        _CACHE["nc"] = _build()
    return _CACHE["nc"]


def _prep(C, Q, W0, c_mask, q_mask):
    C = np.ascontiguousarray(np.asarray(C, dtype=np.float32))
    Q = np.ascontiguousarray(np.asarray(Q, dtype=np.float32))
    W0 = np.ascontiguousarray(np.asarray(W0, dtype=np.float32))
    cm = np.asarray(c_mask).astype(np.float32)
    qm = np.asarray(q_mask).astype(np.float32)
    wq = W0[D:2 * D]
    vq = ((Q.reshape(-1, D) @ wq).reshape(B, LQ) + NEGB * qm).astype(np.float32)
    msp = ((1.0 - cm) * SP).astype(np.float32)
    Cb = np.ascontiguousarray(C.astype(ml_dtypes.bfloat16))
    C8 = np.ascontiguousarray((C * SC8).astype(ml_dtypes.float8_e4m3))
    Qb = np.ascontiguousarray(Q.astype(ml_dtypes.bfloat16))
    return C, Cb, C8, Qb, W0, vq, msp


def kernel(C, Q, W0, c_mask, q_mask):
    nc = _get_nc()
    C, Cb, C8, Qb, W0, vq, msp = _prep(C, Q, W0, c_mask, q_mask)
    in_maps = []
    for c in range(NCORES):
        s = slice(c * BPC, (c + 1) * BPC)
        in_maps.append({"C": C[s], "Cb": Cb[s], "C8": C8[s], "Qb": Qb[s],
                        "W0": W0, "msp": msp[s], "vq": vq[s]})
    res = run_bass_kernel_spmd(nc, in_maps, core_ids=list(range(NCORES)))
    out = np.concatenate([res.results[c]["out"] for c in range(NCORES)], axis=0)
    return out


if __name__ == "__main__":
    sys.path.insert(0, "/root/problem")
    d = np.load("/tmp/ref_cache.npz")
    inputs = {k: d[k] for k in ("C", "Q", "W0", "c_mask", "q_mask")}
    expected = d["expected"]
    actual = kernel(**inputs)
    err = np.abs(actual - expected)
    denom = np.abs(expected).max()
    print("max abs err:", err.max(), "rel:", err.max() / denom)
    for i, name in enumerate(["C", "A", "C*A", "C*Bt"]):
        sl = err[:, :, i * 512:(i + 1) * 512]
        print(f"  {name}: rel {sl.max() / denom:.4e}")


# revision 4
# speedup vs baseline: 1.1551x; 1.1551x over previous
"""Trainium2 Bass kernel for ContextQueryAttention (BiDAF-style), v4.

v3 mixed-precision scheme + off-chip dtype staging:
  - host ships C (f32), Cb=bf16(C), C8=fp8(16C), Qb=bf16(Q), msp, vq
  - CT/QT via XBAR DMA-transpose directly from DRAM bf16 (no PE transposes,
    no PSUM->SBUF copies for them, no on-device converts of C/Q)
  - scores/A in bf16, T/B/c0 in fp8 DoubleRow, P fp8 masked copy for the
    column path; epilogue split across Pool/ACT/DVE.
  - merged DMAs (2-4 m-tiles per transfer) to cut dispatch overhead.
"""
import sys
sys.path.insert(0, "/opt/trn_rl_repo")

import numpy as np
import ml_dtypes
from contextlib import ExitStack

from concourse import bass, bacc, mybir, tile, masks
from concourse.bass_utils import run_bass_kernel_spmd

F32 = mybir.dt.float32
BF16 = mybir.dt.bfloat16
F8 = mybir.dt.float8e4
AF = mybir.ActivationFunctionType
OP = mybir.AluOpType
DR = mybir.MatmulPerfMode.DoubleRow

B, LC, LQ, D = 32, 1024, 256, 512
NCORES = 8
BPC = B // NCORES
MT, JT, KT = LC // 128, LQ // 128, D // 128   # 8, 2, 4
NEGB = -30.0
SP = 0.5        # P fp8 scale
SC8 = 16.0      # C fp8 scale
EPS = 1e-20

_CACHE = {}


def _build():
    nc = bacc.Bacc("TRN2", target_bir_lowering=False, debug=False)
    C_d = nc.dram_tensor("C", [BPC, LC, D], F32, kind="ExternalInput")
    Cb_d = nc.dram_tensor("Cb", [BPC, LC, D], BF16, kind="ExternalInput")
    C8_d = nc.dram_tensor("C8", [BPC, LC, D], F8, kind="ExternalInput")
    Qb_d = nc.dram_tensor("Qb", [BPC, LQ, D], BF16, kind="ExternalInput")
    W_d = nc.dram_tensor("W0", [3 * D], F32, kind="ExternalInput")
    msp_d = nc.dram_tensor("msp", [BPC, LC], F32, kind="ExternalInput")
    vq_d = nc.dram_tensor("vq", [BPC, LQ], F32, kind="ExternalInput")
    out_d = nc.dram_tensor("out", [BPC, LC, 4 * D], F32, kind="ExternalOutput")

    with tile.TileContext(nc) as tc, ExitStack() as ctx:
        const = ctx.enter_context(tc.tile_pool(name="const", bufs=1))
        big = ctx.enter_context(tc.tile_pool(name="big", bufs=2))
        mid = ctx.enter_context(tc.tile_pool(name="mid", bufs=2))
        sm = ctx.enter_context(tc.tile_pool(name="sm", bufs=4))
        pmm = ctx.enter_context(tc.tile_pool(name="pmm", bufs=3, space="PSUM"))
        psc = ctx.enter_context(tc.tile_pool(name="psc", bufs=2, space="PSUM"))
        ptp = ctx.enter_context(tc.tile_pool(name="ptp", bufs=2, space="PSUM"))
        pc0 = ctx.enter_context(tc.tile_pool(name="pc0", bufs=1, space="PSUM"))

        # ---- constants ----
        W_sb = const.tile([128, 12], F32)      # cols 0:4 wc, 4:8 wq, 8:12 wm
        nc.sync.dma_start(W_sb[:], W_d.ap().rearrange("(n p) -> p n", p=128))
        ident_f = const.tile([128, 128], F32)
        masks.make_identity(nc, ident_f[:])
        ident_bf = const.tile([128, 128], BF16)
        nc.vector.tensor_copy(ident_bf[:], ident_f[:])
        ones_f = const.tile([1, 128], F32)
        nc.gpsimd.memset(ones_f[:], 1.0)
        ones_bf = const.tile([1, 128], BF16)
        nc.vector.tensor_copy(ones_bf[:], ones_f[:])
        ones8f = const.tile([128, 32], F32)
        nc.gpsimd.memset(ones8f[:], 1.0)
        ones8 = const.tile([128, 32], F8)
        nc.vector.tensor_copy(ones8[:], ones8f[:])
        ones8_ap = ones8[:, 0:32].rearrange("p (h x) -> p h x", h=2)[:, :, 0:1]

        for b in range(BPC):
            # ---------------- loads (merged) ----------------
            C_sb = big.tile([128, MT * D], F32, tag="C_sb", bufs=2)
            nc.sync.dma_start(
                C_sb[:].rearrange("p (m d) -> p m d", m=MT),
                C_d.ap()[b].rearrange("(m p) d -> p m d", p=128))
            C8 = big.tile([128, MT * D], F8, tag="C8", bufs=2)
            nc.sync.dma_start(
                C8[:].rearrange("p (m d) -> p m d", m=MT),
                C8_d.ap()[b].rearrange("(m p) d -> p m d", p=128))
            Q_bf = mid.tile([128, JT * D], BF16, tag="Q_bf", bufs=2)
            nc.sync.dma_start(
                Q_bf[:].rearrange("p (j d) -> p j d", j=JT),
                Qb_d.ap()[b].rearrange("(j p) d -> p j d", p=128))
            msp = sm.tile([128, MT], F32, tag="msp")
            nc.sync.dma_start(msp[:], msp_d.ap()[b].rearrange("(m p) -> p m", p=128))
            vqf = sm.tile([1, LQ], F32, tag="vqf")
            nc.sync.dma_start(vqf[:], vq_d.ap()[b].rearrange("(o q) -> o q", o=1))
            vq_bf = sm.tile([1, LQ], BF16, tag="vq_bf")
            nc.vector.tensor_copy(vq_bf[:], vqf[:])

            # ---------------- XBAR transposes from DRAM ----------------
            CT = [big.tile([128, LC], BF16, tag="CT", name=f"CT{_k}", bufs=6)
                  for _k in range(KT)]
            for k in range(KT):
                nc.sync.dma_start(CT[k][:], Cb_d.ap()[b, :, k * 128:(k + 1) * 128],
                                  transpose=True)
            QT = [mid.tile([128, LQ], BF16, tag="QT", name=f"QT{_k}", bufs=6)
                  for _k in range(KT)]
            QW = [mid.tile([128, LQ], BF16, tag="QW", name=f"QW{_k}", bufs=6)
                  for _k in range(KT)]
            for k in range(KT):
                nc.sync.dma_start(QT[k][:], Qb_d.ap()[b, :, k * 128:(k + 1) * 128],
                                  transpose=True)
                nc.vector.tensor_scalar(QW[k][:], QT[k][:],
                                        W_sb[:, 8 + k:9 + k], W_sb[:, k:k + 1],
                                        OP.mult, OP.add)

            # ---------------- scores + exp -> P_bf ----------------
            P_bf = big.tile([128, MT * LQ], BF16, tag="P_bf", bufs=2)
            r_rec = sm.tile([128, MT], F32, tag="r_rec")
            rb_rec = sm.tile([128, MT], F32, tag="rb_rec")
            for m in range(MT):
                ps_S = psc.tile([128, LQ], F32, tag="ps_S")
                for k in range(KT):
                    nc.tensor.matmul(ps_S[:], CT[k][:, m * 128:(m + 1) * 128],
                                     QW[k][:], start=(k == 0), stop=False)
                nc.tensor.matmul(ps_S[:], ones_bf[:], vq_bf[:],
                                 start=False, stop=True, skip_group_check=True)
                r_m = sm.tile([128, 1], F32, tag="r_m", bufs=4)
                nc.scalar.activation(P_bf[:, m * LQ:(m + 1) * LQ], ps_S[:],
                                     AF.Exp, bias=0.0, scale=1.0,
                                     accum_out=r_m[:])
                nc.vector.reciprocal(r_rec[:, m:m + 1], r_m[:])
            nc.vector.tensor_scalar_mul(rb_rec[:], r_rec[:], 1.0 / (SP * 4.0))

            # ---------------- P8m (masked fp8) ----------------
            P8m = big.tile([128, MT * LQ], F8, tag="P8m", bufs=2)
            for m in range(MT):
                nc.vector.tensor_scalar_mul(P8m[:, m * LQ:(m + 1) * LQ],
                                            P_bf[:, m * LQ:(m + 1) * LQ],
                                            msp[:, m:m + 1])

            # ---------------- c0 via DR matvec; T via DR ----------------
            c0q = sm.tile([128, JT], F32, tag="c0q")
            c0r = sm.tile([128, JT], F32, tag="c0r")
            T8 = mid.tile([128, JT * D], F8, tag="T8", bufs=2)
            for jg in range(JT):
                ps_c0 = pc0.tile([128, 1], F32, tag="ps_c0")
                for t in range(MT // 2):
                    lhsT = (P8m[:, t * 512:(t + 1) * 512]
                            .rearrange("p (h j) -> p h j", h=2)
                            [:, :, jg * 128:(jg + 1) * 128])
                    nc.tensor.matmul(ps_c0[:], lhsT, ones8_ap,
                                     start=(t == 0), stop=(t == MT // 2 - 1),
                                     perf_mode=DR)
                # ps_T = SP*SC8*sum(mPC); ps_c0 = SP*sum(mP).
                # T8 = 4*T_true = ps_T * (4/SC8)/ps_c0 -> c0q = 4*ps_c0
                nc.vector.tensor_scalar(c0q[:, jg:jg + 1], ps_c0[:],
                                        4.0, EPS, OP.mult, OP.add)
                nc.vector.reciprocal(c0r[:, jg:jg + 1], c0q[:, jg:jg + 1])
                ps_T = pmm.tile([128, 512], F32, tag="pmm")
                for t in range(MT // 2):
                    lhsT = (P8m[:, t * 512:(t + 1) * 512]
                            .rearrange("p (h j) -> p h j", h=2)
                            [:, :, jg * 128:(jg + 1) * 128])
                    rhs = (C8[:, t * 2 * D:(t + 1) * 2 * D]
                           .rearrange("p (h d) -> p h d", h=2))
                    nc.tensor.matmul(ps_T[:], lhsT, rhs,
                                     start=(t == 0), stop=(t == MT // 2 - 1),
                                     perf_mode=DR)
                nc.vector.tensor_scalar_mul(T8[:, jg * D:(jg + 1) * D],
                                            ps_T[:], c0r[:, jg:jg + 1])

            # ---------------- PT transposes (PE) + PT8 ----------------
            PT = [mid.tile([128, LC], BF16, tag="PT", name=f"PT{_j}", bufs=4)
                  for _j in range(JT)]
            for jg in range(JT):
                for mh in range(2):
                    ps_pt = ptp.tile([128, 512], BF16, tag="ps_tp")
                    for mb in range(4):
                        m = mh * 4 + mb
                        nc.tensor.transpose(
                            ps_pt[:, mb * 128:(mb + 1) * 128],
                            P_bf[:, m * LQ + jg * 128: m * LQ + (jg + 1) * 128],
                            ident_bf[:])
                    nc.vector.tensor_copy(PT[jg][:, mh * 512:(mh + 1) * 512], ps_pt[:])
            PT8 = mid.tile([128, JT * LC], F8, tag="PT8", bufs=2)
            for jg in range(JT):
                nc.vector.tensor_scalar_mul(PT8[:, jg * LC:(jg + 1) * LC],
                                            PT[jg][:], SP)

            # ---------------- A (bf16), Bt (DR), epilogue ----------------
            for t4 in range(2):
                o_st = mid.tile([128, 4 * 1536], F32, tag="o_st", bufs=2)
                for g in range(4):
                    m = 4 * t4 + g
                    o_m = o_st[:, g * 1536:(g + 1) * 1536]
                    ps_A = pmm.tile([128, 512], F32, tag="pmm")
                    for jg in range(JT):
                        nc.tensor.matmul(ps_A[:], PT[jg][:, m * 128:(m + 1) * 128],
                                         Q_bf[:, jg * D:(jg + 1) * D],
                                         start=(jg == 0), stop=(jg == JT - 1))
                    ps_B = pmm.tile([128, 512], F32, tag="pmm")
                    lhsT = (PT8[:].rearrange("p (h i) -> p h i", h=2)
                            [:, :, m * 128:(m + 1) * 128])
                    rhs = T8[:].rearrange("p (h d) -> p h d", h=2)
                    nc.tensor.matmul(ps_B[:], lhsT, rhs, start=True, stop=True,
                                     perf_mode=DR)
                    if m % 2 == 0:
                        nc.vector.tensor_scalar_mul(o_m[:, 0:512], ps_A[:],
                                                    r_rec[:, m:m + 1])
                    else:
                        nc.scalar.activation(o_m[:, 0:512], ps_A[:], AF.Copy,
                                             bias=0.0, scale=r_rec[:, m:m + 1])
                    bt_sb = sm.tile([128, 512], F32, tag="bt_sb", bufs=2)
                    nc.scalar.activation(bt_sb[:], ps_B[:], AF.Copy,
                                         bias=0.0, scale=rb_rec[:, m:m + 1])
                    # C*A' on Pool; C*B' split Pool/DVE
                    nc.gpsimd.tensor_tensor(o_m[:, 512:1024],
                                            C_sb[:, m * D:(m + 1) * D],
                                            o_m[:, 0:512], OP.mult)
                    if m < (3 if b % 2 == 0 else 2):
                        nc.gpsimd.tensor_tensor(o_m[:, 1024:1536],
                                                C_sb[:, m * D:(m + 1) * D],
                                                bt_sb[:], OP.mult)
                    else:
                        nc.vector.tensor_tensor(o_m[:, 1024:1536],
                                                C_sb[:, m * D:(m + 1) * D],
                                                bt_sb[:], OP.mult)
                # merged stores for the 4-m group
                nc.sync.dma_start(
                    out_d.ap()[b, t4 * 512:(t4 + 1) * 512, 0:512]
                    .rearrange("(g p) d -> p g d", p=128),
                    C_sb[:, t4 * 2048:(t4 + 1) * 2048]
                    .rearrange("p (g d) -> p g d", g=4))
                nc.sync.dma_start(
                    out_d.ap()[b, t4 * 512:(t4 + 1) * 512, 512:2048]
                    .rearrange("(g p) c -> p g c", p=128),
                    o_st[:].rearrange("p (g c) -> p g c", g=4))
    nc.compile()
    return nc


def _get_nc():
    if "nc" not in _CACHE:
        _CACHE["nc"] = _build()
    return _CACHE["nc"]


def _prep(C, Q, W0, c_mask, q_mask):
    C = np.ascontiguousarray(np.asarray(C, dtype=np.float32))
    Q = np.ascontiguousarray(np.asarray(Q, dtype=np.float32))
    W0 = np.ascontiguousarray(np.asarray(W0, dtype=np.float32))
    cm = np.asarray(c_mask).astype(np.float32)
    qm = np.asarray(q_mask).astype(np.float32)
    wq = W0[D:2 * D]
    vq = ((Q.reshape(-1, D) @ wq).reshape(B, LQ) + NEGB * qm).astype(np.float32)
    msp = ((1.0 - cm) * SP).astype(np.float32)
    Cb = np.ascontiguousarray(C.astype(ml_dtypes.bfloat16))
    C8 = np.ascontiguousarray((C * SC8).astype(ml_dtypes.float8_e4m3))
    Qb = np.ascontiguousarray(Q.astype(ml_dtypes.bfloat16))
    return C, Cb, C8, Qb, W0, vq, msp


def kernel(C, Q, W0, c_mask, q_mask):
    nc = _get_nc()
    C, Cb, C8, Qb, W0, vq, msp = _prep(C, Q, W0, c_mask, q_mask)
    in_maps = []
    for c in range(NCORES):
        s = slice(c * BPC, (c + 1) * BPC)
        in_maps.append({"C": C[s], "Cb": Cb[s], "C8": C8[s], "Qb": Qb[s],
                        "W0": W0, "msp": msp[s], "vq": vq[s]})
    res = run_bass_kernel_spmd(nc, in_maps, core_ids=list(range(NCORES)))
    out = np.concatenate([res.results[c]["out"] for c in range(NCORES)], axis=0)
    return out


if __name__ == "__main__":
    sys.path.insert(0, "/root/problem")
    d = np.load("/tmp/ref_cache.npz")
    inputs = {k: d[k] for k in ("C", "Q", "W0", "c_mask", "q_mask")}
    expected = d["expected"]
    actual = kernel(**inputs)
    err = np.abs(actual - expected)
    denom = np.abs(expected).max()
    print("max abs err:", err.max(), "rel:", err.max() / denom)
    for i, name in enumerate(["C", "A", "C*A", "C*Bt"]):
        sl = err[:, :, i * 512:(i + 1) * 512]
        print(f"  {name}: rel {sl.max() / denom:.4e}")


# revision 7
# speedup vs baseline: 1.2370x; 1.0709x over previous
"""Trainium2 Bass kernel for ContextQueryAttention (BiDAF-style), v4.

v3 mixed-precision scheme + off-chip dtype staging:
  - host ships C (f32), Cb=bf16(C), C8=fp8(16C), Qb=bf16(Q), msp, vq
  - CT/QT via XBAR DMA-transpose directly from DRAM bf16 (no PE transposes,
    no PSUM->SBUF copies for them, no on-device converts of C/Q)
  - scores/A in bf16, T/B/c0 in fp8 DoubleRow, P fp8 masked copy for the
    column path; epilogue split across Pool/ACT/DVE.
  - merged DMAs (2-4 m-tiles per transfer) to cut dispatch overhead.
"""
import sys
sys.path.insert(0, "/opt/trn_rl_repo")

import numpy as np
import ml_dtypes
from contextlib import ExitStack

from concourse import bass, bacc, mybir, tile, masks
from concourse.bass_utils import run_bass_kernel_spmd

F32 = mybir.dt.float32
BF16 = mybir.dt.bfloat16
F8 = mybir.dt.float8e4
AF = mybir.ActivationFunctionType
OP = mybir.AluOpType
DR = mybir.MatmulPerfMode.DoubleRow

B, LC, LQ, D = 32, 1024, 256, 512
NCORES = 8
BPC = B // NCORES
MT, JT, KT = LC // 128, LQ // 128, D // 128   # 8, 2, 4
NEGB = -30.0
SP = 0.5        # P fp8 scale
SC8 = 16.0      # C fp8 scale
EPS = 1e-20

_CACHE = {}


def _build():
    nc = bacc.Bacc("TRN2", target_bir_lowering=False, debug=False)
    C_d = nc.dram_tensor("C", [BPC, LC, D], F32, kind="ExternalInput")
    Cb_d = nc.dram_tensor("Cb", [BPC, LC, D], BF16, kind="ExternalInput")
    C8_d = nc.dram_tensor("C8", [BPC, LC, D], F8, kind="ExternalInput")
    Qb_d = nc.dram_tensor("Qb", [BPC, LQ, D], BF16, kind="ExternalInput")
    W_d = nc.dram_tensor("W0", [3 * D], F32, kind="ExternalInput")
    msp_d = nc.dram_tensor("msp", [BPC, LC], F32, kind="ExternalInput")
    vq_d = nc.dram_tensor("vq", [BPC, LQ], F32, kind="ExternalInput")
    out_d = nc.dram_tensor("out", [BPC, LC, 4 * D], F32, kind="ExternalOutput")

    with tile.TileContext(nc) as tc, ExitStack() as ctx:
        const = ctx.enter_context(tc.tile_pool(name="const", bufs=1))
        big = ctx.enter_context(tc.tile_pool(name="big", bufs=2))
        mid = ctx.enter_context(tc.tile_pool(name="mid", bufs=2))
        sm = ctx.enter_context(tc.tile_pool(name="sm", bufs=4))
        pmm = ctx.enter_context(tc.tile_pool(name="pmm", bufs=3, space="PSUM"))
        psc = ctx.enter_context(tc.tile_pool(name="psc", bufs=2, space="PSUM"))
        ptp = ctx.enter_context(tc.tile_pool(name="ptp", bufs=2, space="PSUM"))
        pc0 = ctx.enter_context(tc.tile_pool(name="pc0", bufs=1, space="PSUM"))

        # ---- constants ----
        W_sb = const.tile([128, 12], F32)      # cols 0:4 wc, 4:8 wq, 8:12 wm
        nc.sync.dma_start(W_sb[:], W_d.ap().rearrange("(n p) -> p n", p=128))
        ident_f = const.tile([128, 128], F32)
        masks.make_identity(nc, ident_f[:])
        ident_bf = const.tile([128, 128], BF16)
        nc.vector.tensor_copy(ident_bf[:], ident_f[:])
        ones_f = const.tile([1, 128], F32)
        nc.gpsimd.memset(ones_f[:], 1.0)
        ones_bf = const.tile([1, 128], BF16)
        nc.vector.tensor_copy(ones_bf[:], ones_f[:])
        ones8f = const.tile([128, 32], F32)
        nc.gpsimd.memset(ones8f[:], 1.0)
        ones8 = const.tile([128, 32], F8)
        nc.vector.tensor_copy(ones8[:], ones8f[:])
        ones8_ap = ones8[:, 0:32].rearrange("p (h x) -> p h x", h=2)[:, :, 0:1]

        for b in range(BPC):
            # ---------------- loads, ordered by first use ----------------
            # scores need CT/QW first; C8 feeds T (mid); C_sb only the epilogue
            msp = sm.tile([128, MT], F32, tag="msp")
            nc.sync.dma_start(msp[:], msp_d.ap()[b].rearrange("(m p) -> p m", p=128))
            vqf = sm.tile([1, LQ], F32, tag="vqf")
            nc.sync.dma_start(vqf[:], vq_d.ap()[b].rearrange("(o q) -> o q", o=1))
            vq_bf = sm.tile([1, LQ], BF16, tag="vq_bf")
            nc.vector.tensor_copy(vq_bf[:], vqf[:])
            CT = [big.tile([128, LC], BF16, tag="CT", name=f"CT{_k}", bufs=6)
                  for _k in range(KT)]
            for k in range(KT):
                nc.sync.dma_start(CT[k][:], Cb_d.ap()[b, :, k * 128:(k + 1) * 128],
                                  transpose=True)
            QT = [mid.tile([128, LQ], BF16, tag="QT", name=f"QT{_k}", bufs=6)
                  for _k in range(KT)]
            QW = [mid.tile([128, LQ], BF16, tag="QW", name=f"QW{_k}", bufs=6)
                  for _k in range(KT)]
            for k in range(KT):
                nc.sync.dma_start(QT[k][:], Qb_d.ap()[b, :, k * 128:(k + 1) * 128],
                                  transpose=True)
                nc.vector.tensor_scalar(QW[k][:], QT[k][:],
                                        W_sb[:, 8 + k:9 + k], W_sb[:, k:k + 1],
                                        OP.mult, OP.add)
            Q_bf = mid.tile([128, JT * D], BF16, tag="Q_bf", bufs=2)
            nc.sync.dma_start(
                Q_bf[:].rearrange("p (j d) -> p j d", j=JT),
                Qb_d.ap()[b].rearrange("(j p) d -> p j d", p=128))
            C8 = big.tile([128, MT * D], F8, tag="C8", bufs=2)
            nc.sync.dma_start(
                C8[:].rearrange("p (m d) -> p m d", m=MT),
                C8_d.ap()[b].rearrange("(m p) d -> p m d", p=128))
            C_sb = big.tile([128, MT * D], F32, tag="C_sb", bufs=2)
            nc.sync.dma_start(
                C_sb[:].rearrange("p (m d) -> p m d", m=MT),
                C_d.ap()[b].rearrange("(m p) d -> p m d", p=128))

            # ---------------- scores + exp -> P_bf ----------------
            P_bf = big.tile([128, MT * LQ], BF16, tag="P_bf", bufs=2)
            r_rec = sm.tile([128, MT], F32, tag="r_rec")
            rb_rec = sm.tile([128, MT], F32, tag="rb_rec")
            for m in range(MT):
                ps_S = psc.tile([128, LQ], F32, tag="ps_S")
                for k in range(KT):
                    nc.tensor.matmul(ps_S[:], CT[k][:, m * 128:(m + 1) * 128],
                                     QW[k][:], start=(k == 0), stop=False)
                nc.tensor.matmul(ps_S[:], ones_bf[:], vq_bf[:],
                                 start=False, stop=True, skip_group_check=True)
                r_m = sm.tile([128, 1], F32, tag="r_m", bufs=4)
                nc.scalar.activation(P_bf[:, m * LQ:(m + 1) * LQ], ps_S[:],
                                     AF.Exp, bias=0.0, scale=1.0,
                                     accum_out=r_m[:])
                nc.vector.reciprocal(r_rec[:, m:m + 1], r_m[:])
            nc.vector.tensor_scalar_mul(rb_rec[:], r_rec[:], 1.0 / (SP * 4.0))

            # ---------------- P8m (masked fp8) ----------------
            P8m = big.tile([128, MT * LQ], F8, tag="P8m", bufs=2)
            for m in range(MT):
                nc.vector.tensor_scalar_mul(P8m[:, m * LQ:(m + 1) * LQ],
                                            P_bf[:, m * LQ:(m + 1) * LQ],
                                            msp[:, m:m + 1])

            # ---------------- c0 via DR matvec; T via DR ----------------
            c0q = sm.tile([128, JT], F32, tag="c0q")
            c0r = sm.tile([128, JT], F32, tag="c0r")
            T8 = mid.tile([128, JT * D], F8, tag="T8", bufs=2)
            for jg in range(JT):
                ps_c0 = pc0.tile([128, 1], F32, tag="ps_c0")
                for t in range(MT // 2):
                    lhsT = (P8m[:, t * 512:(t + 1) * 512]
                            .rearrange("p (h j) -> p h j", h=2)
                            [:, :, jg * 128:(jg + 1) * 128])
                    nc.tensor.matmul(ps_c0[:], lhsT, ones8_ap,
                                     start=(t == 0), stop=(t == MT // 2 - 1),
                                     perf_mode=DR)
                # ps_T = SP*SC8*sum(mPC); ps_c0 = SP*sum(mP).
                # T8 = 4*T_true = ps_T * (4/SC8)/ps_c0 -> c0q = 4*ps_c0
                nc.vector.tensor_scalar(c0q[:, jg:jg + 1], ps_c0[:],
                                        4.0, EPS, OP.mult, OP.add)
                nc.vector.reciprocal(c0r[:, jg:jg + 1], c0q[:, jg:jg + 1])
                ps_T = pmm.tile([128, 512], F32, tag="pmm")
                for t in range(MT // 2):
                    lhsT = (P8m[:, t * 512:(t + 1) * 512]
                            .rearrange("p (h j) -> p h j", h=2)
                            [:, :, jg * 128:(jg + 1) * 128])
                    rhs = (C8[:, t * 2 * D:(t + 1) * 2 * D]
                           .rearrange("p (h d) -> p h d", h=2))
                    nc.tensor.matmul(ps_T[:], lhsT, rhs,
                                     start=(t == 0), stop=(t == MT // 2 - 1),
                                     perf_mode=DR)
                nc.vector.tensor_scalar_mul(T8[:, jg * D:(jg + 1) * D],
                                            ps_T[:], c0r[:, jg:jg + 1])

            # ---------------- PT transposes (PE) + PT8 ----------------
            PT = [mid.tile([128, LC], BF16, tag="PT", name=f"PT{_j}", bufs=4)
                  for _j in range(JT)]
            for jg in range(JT):
                for mh in range(2):
                    ps_pt = ptp.tile([128, 512], BF16, tag="ps_tp")
                    for mb in range(4):
                        m = mh * 4 + mb
                        nc.tensor.transpose(
                            ps_pt[:, mb * 128:(mb + 1) * 128],
                            P_bf[:, m * LQ + jg * 128: m * LQ + (jg + 1) * 128],
                            ident_bf[:])
                    nc.vector.tensor_copy(PT[jg][:, mh * 512:(mh + 1) * 512], ps_pt[:])
            PT8 = mid.tile([128, JT * LC], F8, tag="PT8", bufs=2)
            for jg in range(JT):
                nc.vector.tensor_scalar_mul(PT8[:, jg * LC:(jg + 1) * LC],
                                            PT[jg][:], SP)

            # ---------------- A (bf16), Bt (DR), epilogue ----------------
            for t4 in range(2):
                o_st = mid.tile([128, 4 * 1536], F32, tag="o_st", bufs=2)
                for g in range(4):
                    m = 4 * t4 + g
                    o_m = o_st[:, g * 1536:(g + 1) * 1536]
                    ps_A = pmm.tile([128, 512], F32, tag="pmm")
                    for jg in range(JT):
                        nc.tensor.matmul(ps_A[:], PT[jg][:, m * 128:(m + 1) * 128],
                                         Q_bf[:, jg * D:(jg + 1) * D],
                                         start=(jg == 0), stop=(jg == JT - 1))
                    ps_B = pmm.tile([128, 512], F32, tag="pmm")
                    lhsT = (PT8[:].rearrange("p (h i) -> p h i", h=2)
                            [:, :, m * 128:(m + 1) * 128])
                    rhs = T8[:].rearrange("p (h d) -> p h d", h=2)
                    nc.tensor.matmul(ps_B[:], lhsT, rhs, start=True, stop=True,
                                     perf_mode=DR)
                    if m % 2 == 0:
                        nc.vector.tensor_scalar_mul(o_m[:, 0:512], ps_A[:],
                                                    r_rec[:, m:m + 1])
                    else:
                        nc.scalar.activation(o_m[:, 0:512], ps_A[:], AF.Copy,
                                             bias=0.0, scale=r_rec[:, m:m + 1])
                    bt_sb = sm.tile([128, 512], F32, tag="bt_sb", bufs=2)
                    nc.scalar.activation(bt_sb[:], ps_B[:], AF.Copy,
                                         bias=0.0, scale=rb_rec[:, m:m + 1])
                    # C*A' on Pool; C*B' split Pool/DVE
                    nc.gpsimd.tensor_tensor(o_m[:, 512:1024],
                                            C_sb[:, m * D:(m + 1) * D],
                                            o_m[:, 0:512], OP.mult)
                    if m < (3 if b % 2 == 0 else 2):
                        nc.gpsimd.tensor_tensor(o_m[:, 1024:1536],
                                                C_sb[:, m * D:(m + 1) * D],
                                                bt_sb[:], OP.mult)
                    else:
                        nc.vector.tensor_tensor(o_m[:, 1024:1536],
                                                C_sb[:, m * D:(m + 1) * D],
                                                bt_sb[:], OP.mult)
                # merged stores for the 4-m group
                nc.sync.dma_start(
                    out_d.ap()[b, t4 * 512:(t4 + 1) * 512, 0:512]
                    .rearrange("(g p) d -> p g d", p=128),
                    C_sb[:, t4 * 2048:(t4 + 1) * 2048]
                    .rearrange("p (g d) -> p g d", g=4))
                nc.sync.dma_start(
                    out_d.ap()[b, t4 * 512:(t4 + 1) * 512, 512:2048]
                    .rearrange("(g p) c -> p g c", p=128),
                    o_st[:].rearrange("p (g c) -> p g c", g=4))
    nc.compile()
    return nc


def _get_nc():
    if "nc" not in _CACHE:
        _CACHE["nc"] = _build()
    return _CACHE["nc"]


def _prep(C, Q, W0, c_mask, q_mask):
    C = np.ascontiguousarray(np.asarray(C, dtype=np.float32))
    Q = np.ascontiguousarray(np.asarray(Q, dtype=np.float32))
    W0 = np.ascontiguousarray(np.asarray(W0, dtype=np.float32))
    cm = np.asarray(c_mask).astype(np.float32)
    qm = np.asarray(q_mask).astype(np.float32)
    wq = W0[D:2 * D]
    vq = ((Q.reshape(-1, D) @ wq).reshape(B, LQ) + NEGB * qm).astype(np.float32)
    msp = ((1.0 - cm) * SP).astype(np.float32)
    Cb = np.ascontiguousarray(C.astype(ml_dtypes.bfloat16))
    C8 = np.ascontiguousarray((C * SC8).astype(ml_dtypes.float8_e4m3))
    Qb = np.ascontiguousarray(Q.astype(ml_dtypes.bfloat16))
    return C, Cb, C8, Qb, W0, vq, msp


def kernel(C, Q, W0, c_mask, q_mask):
    nc = _get_nc()
    C, Cb, C8, Qb, W0, vq, msp = _prep(C, Q, W0, c_mask, q_mask)
    in_maps = []
    for c in range(NCORES):
        s = slice(c * BPC, (c + 1) * BPC)
        in_maps.append({"C": C[s], "Cb": Cb[s], "C8": C8[s], "Qb": Qb[s],
                        "W0": W0, "msp": msp[s], "vq": vq[s]})
    res = run_bass_kernel_spmd(nc, in_maps, core_ids=list(range(NCORES)))
    out = np.concatenate([res.results[c]["out"] for c in range(NCORES)], axis=0)
    return out


if __name__ == "__main__":
    sys.path.insert(0, "/root/problem")
    d = np.load("/tmp/ref_cache.npz")
    inputs = {k: d[k] for k in ("C", "Q", "W0", "c_mask", "q_mask")}
    expected = d["expected"]
    actual = kernel(**inputs)
    err = np.abs(actual - expected)
    denom = np.abs(expected).max()
    print("max abs err:", err.max(), "rel:", err.max() / denom)
    for i, name in enumerate(["C", "A", "C*A", "C*Bt"]):
        sl = err[:, :, i * 512:(i + 1) * 512]
        print(f"  {name}: rel {sl.max() / denom:.4e}")
